# revision 1
# baseline (speedup 1.0000x reference)
"""EnsembleFC (E=16 MLPs, 512->512->512->1, relu) on 8 TRN2 NeuronCores.

Strategy (expert parallel): each core owns E/8 = 2 ensemble members' weights
and computes their [B] output column; x is replicated. All activations stay
in "feature-major" (transposed) layout so no on-device transposes are needed:

    h1^T = relu(W1^T @ x^T + b1)      [H, B]
    h2^T = relu(W2^T @ h1^T + b2)     [H, B]
    out^T = W3^T @ h2^T               [1, B]   (b3 added on host)

Matmuls run in float32r (TRN2 reduced-precision fp32 PE mode, 1 cycle/row --
4x faster than plain fp32, ~20x more accurate than bf16; measured scaled
error ~1.5e-4 per 128-deep contraction with raw fp32 inputs).

Raw Bass (no Tile framework): this container's walrus rejects instructions
with more than a couple of sync waits, which Tile's auto-generated drains
exceed. Explicit per-engine programs with standalone waits keep every
instruction at one wait.

Pipeline per chunk of 512 batch columns (PE order, software-pipelined):
  ... L1(c, interleaved members) L3(c-1,m0) L3(c-1,m1) L2(c, interleaved) ...
  Members' output-tile groups interleave in pairs (ORDER) so the relu that
  recycles one member's psum banks lands while the other member's groups
  occupy the PE -- doubles the bank-drain slack.
  PSUM: each member-layer pair owns 2 banks (mt % 2 rotation); L3 reuses the
       member's second L2 bank at partition 0.
  ACT: relu+bias drains psum into h1/h2 (f32r).
  DVE: reduces h2 over k-tiles with the w3 weights in exact fp32
       (t_r = sum_kt w3[kt] * h2[kt], rounded to f32r at the end), so L3 is a
       single ones-vector matmul per member-chunk instead of four; also
       copies L3 psum rows to the output staging buffer.
  SP:  weight DMAs (per-tensor sems, split per k-tile), x chunk DMAs
       (per-slot sems -- DMA queue completions are unordered), output stores.
A short burst of dummy matmuls on scratch SBUF during the DMA prologue keeps
the PE HAM clock-gate warm so chunk 0 runs at full clock.
"""
import numpy as np

E, D, H, B = 16, 512, 512, 8192
N_CORES = 8
MPC = E // N_CORES          # members per core
KT = D // 128               # k-tiles per 512 contraction
MT = H // 128               # m-tiles per 512 output dim
CH = 512                    # batch columns per chunk (one psum bank)
NCH = B // CH               # chunks
XBUF = 4                    # x chunk buffering

_CACHE = {}


def _build():
    import concourse.bass as bass
    from concourse import mybir

    f32 = mybir.dt.float32
    f32r = mybir.dt.float32r

    nc = bass.Bass("TRN2", target_bir_lowering=False, debug=False,
                   num_devices=N_CORES)

    xT = nc.dram_tensor("xT", [D, B], f32r, kind="ExternalInput").ap()
    w1 = nc.dram_tensor("w1", [MPC, D, H], f32r, kind="ExternalInput").ap()
    w2 = nc.dram_tensor("w2", [MPC, H, H], f32r, kind="ExternalInput").ap()
    # host-side pre-arranged: w3[p, m, kt], b1/b2[p, m, mt]
    w3 = nc.dram_tensor("w3", [128, MPC, KT], f32r, kind="ExternalInput").ap()
    b1 = nc.dram_tensor("b1", [128, MPC, MT], f32, kind="ExternalInput").ap()
    b2 = nc.dram_tensor("b2", [128, MPC, MT], f32, kind="ExternalInput").ap()
    one = nc.dram_tensor("one", [128, 1], f32r, kind="ExternalInput").ap()
    out = nc.dram_tensor("out", [MPC, B], f32, kind="ExternalOutput").ap()

    w1s = [nc.alloc_sbuf_tensor(f"w1s{m}", [128, KT, H], f32r).ap()
           for m in range(MPC)]
    w2s = [nc.alloc_sbuf_tensor(f"w2s{m}", [128, KT, H], f32r).ap()
           for m in range(MPC)]
    w3s = nc.alloc_sbuf_tensor("w3s", [128, MPC, KT], f32r).ap()
    b1s = nc.alloc_sbuf_tensor("b1s", [128, MPC, MT], f32).ap()
    b2s = nc.alloc_sbuf_tensor("b2s", [128, MPC, MT], f32).ap()
    ones_s = nc.alloc_sbuf_tensor("ones_s", [128, 1], f32r).ap()
    xs = nc.alloc_sbuf_tensor("xs", [128, XBUF, KT, CH], f32r).ap()
    h1 = nc.alloc_sbuf_tensor("h1", [128, MPC, KT, CH], f32r).ap()
    h2 = nc.alloc_sbuf_tensor("h2", [128, MPC, KT, CH], f32r).ap()
    # DVE kt-reduction scratch (no aliasing: A,B pair-products, C,D partials)
    rA = nc.alloc_sbuf_tensor("rA", [128, CH], f32).ap()
    rB = nc.alloc_sbuf_tensor("rB", [128, CH], f32).ap()
    rC = nc.alloc_sbuf_tensor("rC", [128, CH], f32).ap()
    rD = nc.alloc_sbuf_tensor("rD", [128, CH], f32).ap()
    rE = nc.alloc_sbuf_tensor("rE", [128, CH], f32).ap()
    rF = nc.alloc_sbuf_tensor("rF", [128, CH], f32).ap()
    t_r = nc.alloc_sbuf_tensor("t_r", [128, MPC, CH], f32r).ap()
    # per-member output staging, both at partition 0
    osb = [nc.alloc_sbuf_tensor(f"osb{m}", [1, NCH, CH], f32).ap()
           for m in range(MPC)]

    psA = nc.alloc_psum_tensor("psA", [128, 2 * MPC, CH], f32).ap()  # L1
    psB = nc.alloc_psum_tensor("psB", [128, 2 * MPC, CH], f32).ap()  # L2+L3

    # PE warmup scratch: dummy matmuls during the DMA prologue keep the HAM
    # clock-gate ramp off the critical path (uninitialized on HW -- harmless)
    scr = nc.alloc_sbuf_tensor("scr", [128, 128 + CH], f32r).ap()
    N_WARM = _CACHE.get("n_warm_override", 28)

    xT_r = xT.rearrange("(kt p) b -> p kt b", p=128)

    # --- tick tables (absolute semaphore counts, mirror emission order) ---
    # members' groups interleave in pairs so the bank-drain relu of one
    # member lands while the other member's groups occupy the PE
    ORDER = [(0, 0), (0, 1), (1, 0), (1, 1), (0, 2), (0, 3), (1, 2), (1, 3)]
    mmT = {}
    _t = 0
    for c in range(NCH):
        for m, mt in ORDER:
            _t += 1
            mmT[("l1", c, m, mt)] = _t
        if c >= 1:
            for m in range(MPC):
                _t += 1
                mmT[("l3", c - 1, m)] = _t
        for m, mt in ORDER:
            _t += 1
            mmT[("l2", c, m, mt)] = _t
    for m in range(MPC):
        _t += 1
        mmT[("l3", NCH - 1, m)] = _t

    actT = {}
    _a = 0
    for c in range(NCH):
        for m, mt in ORDER:
            _a += 1
            actT[("r1", c, m, mt)] = _a
        for m, mt in ORDER:
            _a += 1
            actT[("r2", c, m, mt)] = _a

    def act_r1(c, m, mt):
        return actT[("r1", c, m, mt)]

    def act_r2(c, m, mt):
        return actT[("r2", c, m, mt)]

    # DVE tick table: per chunk red(m0), red(m1) [skipped for the last
    # chunk -- its L3 runs directly off h2], then cp(m0), cp(m1)
    dveT = {}
    _d = 0
    for c in range(NCH):
        if c < NCH - 1:
            for m in range(MPC):
                _d += 1
                dveT[("red", c, m)] = _d
        for m in range(MPC):
            _d += 1
            dveT[("cp", c, m)] = _d

    def dve_red(c, m):
        return dveT[("red", c, m)]

    def dve_cp(c, m):
        return dveT[("cp", c, m)]

    with (
        nc.Block() as block,
        nc.semaphore("mm_sem") as mm_sem,
        nc.semaphore("act_sem") as act_sem,
        nc.semaphore("b1_sem") as b1_sem,
        nc.semaphore("b2_sem") as b2_sem,
        nc.semaphore("w3_sem") as w3_sem,
        nc.semaphore("d_sem") as d_sem,
    ):
        # per-slot x semaphores: DMA queue completions are unordered across
        # chunks, so a single cumulative counter would be racy
        x_sems = [nc.alloc_semaphore(f"x_sem{s}") for s in range(XBUF)]
        dve_sem = nc.alloc_semaphore("dve_sem")
        rd_sem = nc.alloc_semaphore("rd_sem")   # intra-DVE RAW/WAR ordering
        w1_sems = [nc.alloc_semaphore(f"w1_sem{m}") for m in range(MPC)]
        w2_sems = [nc.alloc_semaphore(f"w2_sem{m}") for m in range(MPC)]
        # member-0 W1 arrives per output-tile: chunk 0's first groups start
        # after x0 + one 256KB weight slice instead of x0 + 1MB
        w1m_sems = [nc.alloc_semaphore(f"w1m_sem{t}") for t in range(MT)]

        def dma_x(sync, c):
            for kt in range(KT):
                sync.dma_start(
                    out=xs[:, c % XBUF, kt, :],
                    in_=xT_r[:, kt, c * CH:(c + 1) * CH],
                ).then_inc(x_sems[c % XBUF], 16)

        @block.sync
        def _(sync: bass.BassEngine):
            # interleave weight loads with early x chunks, ordered by need
            w1r = [w1[m].rearrange("(kt p) m2 -> p kt m2", p=128)
                   for m in range(MPC)]
            w2r = [w2[m].rearrange("(kt p) m2 -> p kt m2", p=128)
                   for m in range(MPC)]
            for mt in range(MT):
                sync.dma_start(
                    out=w1s[0][:, :, mt * 128:(mt + 1) * 128],
                    in_=w1r[0][:, :, mt * 128:(mt + 1) * 128],
                ).then_inc(w1m_sems[mt], 16)
            sync.dma_start(out=b1s, in_=b1).then_inc(b1_sem, 16)
            dma_x(sync, 0)
            for kt in range(KT):
                sync.dma_start(out=w1s[1][:, kt], in_=w1r[1][:, kt]
                               ).then_inc(w1_sems[1], 16)
            for kt in range(KT):
                sync.dma_start(out=w2s[0][:, kt], in_=w2r[0][:, kt]
                               ).then_inc(w2_sems[0], 16)
            sync.dma_start(out=b2s, in_=b2).then_inc(b2_sem, 16)
            sync.dma_start(out=w3s, in_=w3).then_inc(w3_sem, 16)
            sync.dma_start(out=ones_s, in_=one).then_inc(w3_sem, 16)
            dma_x(sync, 1)
            for kt in range(KT):
                sync.dma_start(out=w2s[1][:, kt], in_=w2r[1][:, kt]
                               ).then_inc(w2_sems[1], 16)
            dma_x(sync, 2)
            dma_x(sync, 3)

            out_r = out.rearrange("m (nch ch) -> m nch ch", ch=CH)
            for c in range(XBUF, NCH):
                # x slot free once L1 of chunk c-XBUF fully consumed it
                sync.wait_ge(mm_sem, mmT[("l1", c - XBUF, MPC - 1, MT - 1)])
                dma_x(sync, c)
                # trailing store for chunk c-XBUF
                cs = c - XBUF
                sync.wait_ge(dve_sem, dve_cp(cs, MPC - 1))
                for m in range(MPC):
                    sync.dma_start(out=out_r[m:m + 1, cs],
                                   in_=osb[m][:, cs]).then_inc(d_sem, 16)

            for cs in range(NCH - XBUF, NCH):
                sync.wait_ge(dve_sem, dve_cp(cs, MPC - 1))
                for m in range(MPC):
                    sync.dma_start(out=out_r[m:m + 1, cs],
                                   in_=osb[m][:, cs]).then_inc(d_sem, 16)
            sync.wait_ge(d_sem, 16 * MPC * NCH)

        @block.vector
        def _(vector: bass.BassEngine):
            # DVE: (a) kt-reduction t_r = sum_kt w3[kt]*h2[kt] in exact fp32
            # (takes 3 of every 4 L3 matmuls off the PE, and is more accurate
            # than f32r products), (b) L3 psum -> osb copies.
            w3f = w3s.bitcast(f32)
            vector.wait_ge(w3_sem, 32)   # w3s + ones loaded
            for c in range(NCH):
                for m in range(MPC):
                    if c == NCH - 1:
                        break   # last chunk: PE computes L3 directly
                    # h2 ready; implies PE already read t_r(c-1, m) (its L3
                    # precedes this chunk's L2 in the PE stream)
                    h2f = h2[:, m].bitcast(f32)
                    # 4 independent muls, each gated on its own relu2, then a
                    # 2-level add tree: the reduction lands ~1 op after the
                    # LAST relu instead of a full serial chain after it.
                    # (DVE does not self-interlock; rd_sem orders RAW/WAR.)
                    base = 6 * (MPC * c + m)
                    if base:
                        vector.wait_ge(rd_sem, base)   # prev group's reads done
                    for kt, buf in enumerate((rA, rB, rC, rD)):
                        vector.wait_ge(act_sem, act_r2(c, m, kt))
                        vector.tensor_scalar_mul(
                            buf, h2f[:, kt, :], w3f[:, m, kt:kt + 1]
                        ).then_inc(rd_sem, 1)
                    vector.wait_ge(rd_sem, base + 2)
                    vector.tensor_add(rE, rA, rB).then_inc(rd_sem, 1)
                    vector.wait_ge(rd_sem, base + 4)
                    vector.tensor_add(rF, rC, rD).then_inc(rd_sem, 1)
                    vector.wait_ge(rd_sem, base + 6)
                    vector.tensor_add(t_r[:, m, :], rE, rF
                                      ).then_inc(dve_sem, 1)
                for m in range(MPC):
                    vector.wait_ge(mm_sem, mmT[("l3", c, m)])
                    vector.tensor_copy(
                        osb[m][0:1, c, :], psB[0:1, 2 * m + 1, :],
                    ).then_inc(dve_sem, 1)

        @block.tensor
        def _(tensor: bass.BassEngine):
            # warmup on uninitialized scratch: values are irrelevant, the psum
            # is overwritten (start=True) before any reader
            for i in range(N_WARM):
                tensor.matmul(psA[:, 0, :], scr[:, :128], scr[:, 128:],
                              start=True, stop=True, skip_group_check=True)

            def l3(c, m):
                # single ones-matmul over the DVE-reduced t_r; bank 2m+1 so
                # the osb copy only gates the SECOND L2 group of chunk c+1
                tensor.wait_ge(dve_sem, dve_red(c, m))
                tensor.matmul(
                    psB[0:1, 2 * m + 1, :], ones_s, t_r[:, m, :],
                    start=True, stop=True,
                ).then_inc(mm_sem, 1)

            for c in range(NCH):
                tensor.wait_ge(x_sems[c % XBUF], 64 * (c // XBUF + 1))
                # L1, members interleaved
                for m, mt in ORDER:
                    if mt == 0:
                        if c == 0:
                            if m == 1:
                                tensor.wait_ge(w1_sems[1], 64)
                        else:
                            # banks 2m,2m+1 drained by chunk c-1's L1 relus
                            tensor.wait_ge(act_sem, act_r1(c - 1, m, MT - 1))
                    if c == 0 and m == 0:
                        tensor.wait_ge(w1m_sems[mt], 16)
                    if mt >= 2:               # 2-bank rotation WAR
                        tensor.wait_ge(act_sem, act_r1(c, m, mt - 2))
                    for kt in range(KT):
                        ins = tensor.matmul(
                            psA[:, 2 * m + mt % 2, :],
                            w1s[m][:, kt, mt * 128:(mt + 1) * 128],
                            xs[:, c % XBUF, kt, :],
                            start=(kt == 0), stop=(kt == KT - 1),
                        )
                    ins.then_inc(mm_sem, 1)
                # pipelined L3 of the previous chunk: its DVE reduction ran
                # while this chunk's L1 was on the PE
                if c >= 1:
                    if c == 1:
                        tensor.wait_ge(w3_sem, 32)
                    for m in range(MPC):
                        l3(c - 1, m)
                # L2, members interleaved
                for m, mt in ORDER:
                    if mt == 0:
                        if c == 0:
                            tensor.wait_ge(w2_sems[m], 64)
                        tensor.wait_ge(act_sem, act_r1(c, m, MT - 1))  # h1
                    if mt == 1 and c > 0:
                        # psB bank 2m+1 holds chunk c-1's L3 row until DVE
                        # copies it out
                        tensor.wait_ge(dve_sem, dve_cp(c - 1, m))
                    if mt >= 2:
                        tensor.wait_ge(act_sem, act_r2(c, m, mt - 2))
                    for kt in range(KT):
                        ins = tensor.matmul(
                            psB[:, 2 * m + mt % 2, :],
                            w2s[m][:, kt, mt * 128:(mt + 1) * 128],
                            h1[:, m, kt, :],
                            start=(kt == 0), stop=(kt == KT - 1),
                        )
                    ins.then_inc(mm_sem, 1)
            # tail: direct w3 matmuls for the last chunk -- avoids idling on
            # the serial DVE reduction after the final L2
            for m in range(MPC):
                tensor.wait_ge(act_sem, act_r2(NCH - 1, m, MT - 1))
                for kt in range(KT):
                    ins = tensor.matmul(
                        psB[0:1, 2 * m + 1, :],
                        w3s[:, m, kt:kt + 1],
                        h2[:, m, kt, :],
                        start=(kt == 0), stop=(kt == KT - 1),
                    )
                ins.then_inc(mm_sem, 1)

        @block.scalar
        def _(scalar: bass.BassEngine):
            Relu = bass.mybir.ActivationFunctionType.Relu
            scalar.wait_ge(b1_sem, 16)
            scalar.wait_ge(b2_sem, 16)
            for c in range(NCH):
                for m, mt in ORDER:
                    scalar.wait_ge(mm_sem, mmT[("l1", c, m, mt)])
                    scalar.activation(
                        h1[:, m, mt, :], psA[:, 2 * m + mt % 2, :], Relu,
                        bias=b1s[:, m, mt:mt + 1],
                    ).then_inc(act_sem, 1)
                for m, mt in ORDER:
                    scalar.wait_ge(mm_sem, mmT[("l2", c, m, mt)])
                    scalar.activation(
                        h2[:, m, mt, :], psB[:, 2 * m + mt % 2, :], Relu,
                        bias=b2s[:, m, mt:mt + 1],
                    ).then_inc(act_sem, 1)

    return nc


def get_nc():
    if "nc" not in _CACHE:
        _CACHE["nc"] = _build()
    return _CACHE["nc"]


def kernel(x, W1, b1, W2, b2, W3, b3):
    from concourse.bass_utils import run_bass_kernel_spmd

    nc = get_nc()
    xT = np.ascontiguousarray(np.asarray(x, dtype=np.float32).T)
    W1 = np.asarray(W1, dtype=np.float32)
    W2 = np.asarray(W2, dtype=np.float32)
    W3 = np.asarray(W3, dtype=np.float32)
    b1 = np.asarray(b1, dtype=np.float32)
    b2 = np.asarray(b2, dtype=np.float32)
    b3 = np.asarray(b3, dtype=np.float32)

    def feat_major(v):
        # [MPC, H] -> [128, MPC, H//128]: v[p, m, t] = v_in[m, t*128 + p]
        return np.ascontiguousarray(
            v.reshape(MPC, H // 128, 128).transpose(2, 0, 1))

    in_maps = []
    for c in range(N_CORES):
        s = slice(MPC * c, MPC * (c + 1))
        in_maps.append({
            "xT": xT,
            "w1": np.ascontiguousarray(W1[s]),
            "w2": np.ascontiguousarray(W2[s]),
            "w3": feat_major(W3[s, :, 0]),
            "b1": feat_major(b1[s]),
            "b2": feat_major(b2[s]),
            "one": np.ones((128, 1), dtype=np.float32),
        })

    res = run_bass_kernel_spmd(nc, in_maps, list(range(N_CORES)))
    out = np.concatenate([r["out"] for r in res.results], axis=0)  # [E, B]
    out = out + b3.reshape(E, 1)
    return out.reshape(E, B, 1).astype(np.float32)



# revision 2
# speedup vs baseline: 1.2204x; 1.2204x over previous
"""EnsembleFC (E=16 MLPs, 512->512->512->1, relu) on 8 TRN2 NeuronCores.

Expert-parallel: each core owns E/8 = 2 members; x replicated. Activations
stay feature-major (transposed): h^T = relu(W^T @ x^T + b).

L1 and L2 run as fp8(e4m3) DoubleRow matmuls with an error-compensated
3-plane split. Every operand is stored as a high fp8 tensor plus an fp8
residual at the same scale (v*s = vh + vl + O(eps^2)); a logical product
W^T x then needs three fp8 planes -- Wh.xh + Wh.xl + Wl.xh (the Wl.xl term
is eps^2-small and dropped), all sharing product scale sw*sx so they
accumulate into one psum group. DoubleRow packs 2 such 128-deep planes per
instruction at 0.5 cycles/row, so each logical 128x128x512 tile costs 3/4
of an f32r matmul while the measured end-to-end error stays ~5e-3 scaled
(vs 2e-2 budget). x and W splits are quantized on the host; h1's split is
computed on-device: ACT writes h1h = fp8(relu(psum+b1s)), DVE writes
h1l = fp8(max(psum+b1s,0) - h1h) in one scalar_tensor_tensor op.

Scales: x*4, W1*4 -> psum1 = z1*16; h1 stored at scale 16; W2*4 ->
psum2 = z2*64; h2 stored (f32r) at scale 64; w3 host-scaled by /64 so the
L3 reduction lands at true scale. Biases fold in as b1*16 / b2*64 (ACT
bias APs); b3 added on host.

Engine split per chunk of 512 batch columns (PE ~10.7us is the roofline):
  PE:   8 L1 groups (6 DR matmuls each) | 2 L3 ones-matmuls | 8 L2 groups.
        Members' groups interleave in pairs (ORDER) so one member's psum
        drain lands while the other occupies the PE.
  ACT:  h1h = fp8(relu(psA+b1s)); h2 = relu(psB+b2s) in f32r.
  Pool: member-0's whole L3 kt-reduction (products rA..rD via
        tensor_scalar_mul, add tree, t_r[0]) plus member-1's products
        sA..sD. GPSIMD cannot touch PSUM, so it gets all-SBUF work.
  DVE:  h1l residual; member-1's add tree -> t_r[1]; L3 psum->osb copies.
  SP:   weight/x DMAs (64KB slices so chunk-0's x lands in ~4us across
        parallel queues instead of 11us on one), output stores.

The L3 row for chunk c is produced by the PE slot in chunk c+2 (t_r is
double-buffered): the Pool/DVE reduction gets a whole chunk of slack, so
the PE never stalls on it mid-chunk (a 1-chunk pipeline left ~4.8us/chunk
of PE idle waiting for t_r). L3 of the last chunk runs as direct f32r
matmuls on the PE tail. A short burst of dummy matmuls during the DMA
prologue keeps the PE pstate ramp off the critical path (an idle PE
falls back to half clock for 3us, so gaps are doubly expensive).
"""
import numpy as np
import ml_dtypes

F8NP = ml_dtypes.float8_e4m3

E, D, H, B = 16, 512, 512, 8192
N_CORES = 8
MPC = E // N_CORES          # members per core
KT = D // 128               # k-tiles per 512 contraction
KT2 = KT // 2               # DoubleRow k-tile pairs
MT = H // 128               # m-tiles per 512 output dim
CH = 512                    # batch columns per chunk (one psum bank)
NCH = B // CH               # chunks
XBUF = 4                    # x chunk buffering
SX = 4.0                    # x scale
SW = 4.0                    # weight scale (both layers)
N_WARM = 11                 # PE pstate warmup matmuls

_CACHE = {}


def _build(b1_nonzero):
    import concourse.bass as bass
    from concourse import mybir

    f32 = mybir.dt.float32
    f32r = mybir.dt.float32r
    fp8 = mybir.dt.float8e4
    DR = mybir.MatmulPerfMode.DoubleRow
    Relu = mybir.ActivationFunctionType.Relu
    add_op = mybir.AluOpType.add
    sub_op = mybir.AluOpType.subtract
    max_op = mybir.AluOpType.max

    nc = bass.Bass("TRN2", target_bir_lowering=False, debug=False,
                   num_devices=N_CORES)

    xh = nc.dram_tensor("xh", [D, B], fp8, kind="ExternalInput").ap()
    xl = nc.dram_tensor("xl", [D, B], fp8, kind="ExternalInput").ap()
    w1h = nc.dram_tensor("w1h", [MPC, D, H], fp8, kind="ExternalInput").ap()
    w1l = nc.dram_tensor("w1l", [MPC, D, H], fp8, kind="ExternalInput").ap()
    w2h = nc.dram_tensor("w2h", [MPC, H, H], fp8, kind="ExternalInput").ap()
    w2l = nc.dram_tensor("w2l", [MPC, H, H], fp8, kind="ExternalInput").ap()
    # host-side pre-arranged: w3[p, m, kt] = W3/64, b1[p,m,mt]*16, b2*64
    w3 = nc.dram_tensor("w3", [128, MPC, KT], f32r, kind="ExternalInput").ap()
    b1 = nc.dram_tensor("b1", [128, MPC, MT], f32, kind="ExternalInput").ap()
    b2 = nc.dram_tensor("b2", [128, MPC, MT], f32, kind="ExternalInput").ap()
    one = nc.dram_tensor("one", [128, 1], f32r, kind="ExternalInput").ap()
    out = nc.dram_tensor("out", [MPC, B], f32, kind="ExternalOutput").ap()

    w1hs = [nc.alloc_sbuf_tensor(f"w1hs{m}", [128, KT, H], fp8).ap()
            for m in range(MPC)]
    w1ls = [nc.alloc_sbuf_tensor(f"w1ls{m}", [128, KT, H], fp8).ap()
            for m in range(MPC)]
    w2hs = [nc.alloc_sbuf_tensor(f"w2hs{m}", [128, KT, H], fp8).ap()
            for m in range(MPC)]
    w2ls = [nc.alloc_sbuf_tensor(f"w2ls{m}", [128, KT, H], fp8).ap()
            for m in range(MPC)]
    w3s = nc.alloc_sbuf_tensor("w3s", [128, MPC, KT], f32r).ap()
    b1s = nc.alloc_sbuf_tensor("b1s", [128, MPC, MT], f32).ap()
    b2s = nc.alloc_sbuf_tensor("b2s", [128, MPC, MT], f32).ap()
    ones_s = nc.alloc_sbuf_tensor("ones_s", [128, 1], f32r).ap()
    xsh = nc.alloc_sbuf_tensor("xsh", [128, XBUF, KT, CH], fp8).ap()
    xsl = nc.alloc_sbuf_tensor("xsl", [128, XBUF, KT, CH], fp8).ap()
    h1h = nc.alloc_sbuf_tensor("h1h", [128, MPC, KT, CH], fp8).ap()
    h1l = nc.alloc_sbuf_tensor("h1l", [128, MPC, KT, CH], fp8).ap()
    h2 = nc.alloc_sbuf_tensor("h2", [128, MPC, KT, CH], f32r).ap()
    zero_s = nc.alloc_sbuf_tensor("zero_s", [128, CH], f32).ap()
    if b1_nonzero:
        tb = nc.alloc_sbuf_tensor("tb", [128, CH], f32).ap()
    # L3 kt-reduction scratch: member-0 chain entirely on Pool (rA..rF),
    # member-1 products on Pool (sA..sD), add tree on DVE (sE/sF)
    rA = nc.alloc_sbuf_tensor("rA", [128, CH], f32).ap()
    rB = nc.alloc_sbuf_tensor("rB", [128, CH], f32).ap()
    rC = nc.alloc_sbuf_tensor("rC", [128, CH], f32).ap()
    rD = nc.alloc_sbuf_tensor("rD", [128, CH], f32).ap()
    rE = nc.alloc_sbuf_tensor("rE", [128, CH], f32).ap()
    rF = nc.alloc_sbuf_tensor("rF", [128, CH], f32).ap()
    sA = nc.alloc_sbuf_tensor("sA", [128, CH], f32).ap()
    sB = nc.alloc_sbuf_tensor("sB", [128, CH], f32).ap()
    sC = nc.alloc_sbuf_tensor("sC", [128, CH], f32).ap()
    sD = nc.alloc_sbuf_tensor("sD", [128, CH], f32).ap()
    sE = nc.alloc_sbuf_tensor("sE", [128, CH], f32).ap()
    sF = nc.alloc_sbuf_tensor("sF", [128, CH], f32).ap()
    # t_r double-buffered: written while the PE reads the older chunk's
    t_r = nc.alloc_sbuf_tensor("t_r", [128, MPC, 2, CH], f32r).ap()
    # L3 row staging, both members at partition 0 (engine copies cannot
    # shift partitions)
    osb0 = nc.alloc_sbuf_tensor("osb0", [1, NCH, CH], f32).ap()
    osb1 = nc.alloc_sbuf_tensor("osb1", [1, NCH, CH], f32).ap()

    psA = nc.alloc_psum_tensor("psA", [128, 2 * MPC, CH], f32).ap()  # L1
    psB = nc.alloc_psum_tensor("psB", [128, 2 * MPC, CH], f32).ap()  # L2+L3

    # PE warmup scratch (uninitialized on HW -- harmless, psum start=True)
    scr = nc.alloc_sbuf_tensor("scr", [128, 128 + CH], f32r).ap()

    xh_r = xh.rearrange("(kt p) b -> p kt b", p=128)
    xl_r = xl.rearrange("(kt p) b -> p kt b", p=128)
    w3f = w3s.bitcast(f32)

    # --- tick tables (absolute semaphore counts, mirror emission order) ---
    ORDER = [(0, 0), (0, 1), (1, 0), (1, 1), (0, 2), (0, 3), (1, 2), (1, 3)]
    ORDER_L2 = [(0, 0), (0, 1), (1, 0), (1, 1), (0, 2), (0, 3), (1, 2), (1, 3)]

    mmT = {}
    _t = 0
    for c in range(NCH):
        for m, mt in ORDER:
            _t += 1
            mmT[("l1", c, m, mt)] = _t
        for m, mt in ORDER_L2:
            _t += 1
            mmT[("l2", c, m, mt)] = _t
        if c >= 2:
            for m in range(MPC):
                _t += 1
                mmT[("l3", c - 2, m)] = _t
    for m in range(MPC):
        _t += 1
        mmT[("l3", NCH - 2, m)] = _t
    for m in range(MPC):
        _t += 1
        mmT[("l3", NCH - 1, m)] = _t

    # ACT: per chunk: h1h ORDER | h2 for ORDER6 (member-1 kt 2/3 go to DVE
    # so ACT's backlog never straddles the chunk boundary)
    ORDER6 = [om for om in ORDER_L2 if om not in ((1, 2), (1, 3))]
    actT = {}
    _a = 0
    for c in range(NCH):
        if c >= 3:
            _a += 1
            actT[("cpa", c - 3)] = _a
        for m, mt in ORDER:
            _a += 1
            actT[("r1", c, m, mt)] = _a
        for m, mt in ORDER6:
            _a += 1
            actT[("r2", c, m, mt)] = _a
    for cc in (NCH - 3, NCH - 2, NCH - 1):
        _a += 1
        actT[("cpa", cc)] = _a

    # Pool: chunk cc's reduction runs during chunk cc+1:
    # mul0 k0,k1 | rE | mul0 k2,k3 | rF | t_r[0] | mul1 k0..k3
    # (rE/rF don't inc pool_sem -- no external waiters)
    poolT = {}
    _p = 0
    for cc in range(NCH - 1):
        for kt in range(KT):
            _p += 1
            poolT[("mul0", cc, kt)] = _p
        _p += 1
        poolT[("red0", cc)] = _p
        for kt in range(KT):
            _p += 1
            poolT[("mul1", cc, kt)] = _p

    # DVE: per chunk: cp(c-3) (both rows, one strided op) | h1l ORDER |
    # adds1(c-1) -> red1 | h2d(c, kt=2,3) (member-1 tail h2 tiles)
    dveT = {}
    _d = 0
    for c in range(NCH):
        if c >= 3:
            _d += 1
            dveT[("cp", c - 3)] = _d
        for m, mt in ORDER:
            _d += 1
            dveT[("h1l", c, m, mt)] = _d
        if c >= 1:
            _d += 1
            dveT[("red1", c - 1)] = _d
        for kt in (2, 3):
            _d += 1
            dveT[("h2d", c, kt)] = _d
    for cc in (NCH - 3, NCH - 2, NCH - 1):
        _d += 1
        dveT[("cp", cc)] = _d

    with (
        nc.Block() as block,
        nc.semaphore("mm_sem") as mm_sem,
        nc.semaphore("act_sem") as act_sem,
        nc.semaphore("pool_sem") as pool_sem,
        nc.semaphore("b1_sem") as b1_sem,
        nc.semaphore("b2_sem") as b2_sem,
        nc.semaphore("w3_sem") as w3_sem,
        nc.semaphore("d_sem") as d_sem,
    ):
        x_sems = [nc.alloc_semaphore(f"x_sem{s}") for s in range(XBUF)]
        dve_sem = nc.alloc_semaphore("dve_sem")
        rd2_sem = nc.alloc_semaphore("rd2_sem")  # sE/sF/t_r[1] DVE ordering
        w1_sems = [nc.alloc_semaphore(f"w1_sem{m}") for m in range(MPC)]
        w2_sems = [nc.alloc_semaphore(f"w2_sem{m}") for m in range(MPC)]

        def dma_x(sync, c):
            # one dma_start per tensor: issue bandwidth (~625ns serialized
            # descriptor-gen per dma_start) dominates, not transfer time
            sync.dma_start(
                out=xsh[:, c % XBUF], in_=xh_r[:, :, c * CH:(c + 1) * CH],
            ).then_inc(x_sems[c % XBUF], 16)
            sync.dma_start(
                out=xsl[:, c % XBUF], in_=xl_r[:, :, c * CH:(c + 1) * CH],
            ).then_inc(x_sems[c % XBUF], 16)

        @block.sync
        def _(sync: bass.BassEngine):
            w1hr = [w1h[m].rearrange("(kt p) m2 -> p kt m2", p=128)
                    for m in range(MPC)]
            w1lr = [w1l[m].rearrange("(kt p) m2 -> p kt m2", p=128)
                    for m in range(MPC)]
            w2hr = [w2h[m].rearrange("(kt p) m2 -> p kt m2", p=128)
                    for m in range(MPC)]
            w2lr = [w2l[m].rearrange("(kt p) m2 -> p kt m2", p=128)
                    for m in range(MPC)]
            dma_x(sync, 0)
            sync.dma_start(out=b1s, in_=b1).then_inc(b1_sem, 16)
            sync.dma_start(out=b2s, in_=b2).then_inc(b2_sem, 16)
            sync.dma_start(out=w1hs[0], in_=w1hr[0]).then_inc(w1_sems[0], 16)
            sync.dma_start(out=w1ls[0], in_=w1lr[0]).then_inc(w1_sems[0], 16)
            sync.dma_start(out=w1hs[1], in_=w1hr[1]).then_inc(w1_sems[1], 16)
            sync.dma_start(out=w1ls[1], in_=w1lr[1]).then_inc(w1_sems[1], 16)
            sync.dma_start(out=w2hs[0], in_=w2hr[0]).then_inc(w2_sems[0], 16)
            sync.dma_start(out=w2ls[0], in_=w2lr[0]).then_inc(w2_sems[0], 16)
            sync.dma_start(out=w2hs[1], in_=w2hr[1]).then_inc(w2_sems[1], 16)
            sync.dma_start(out=w2ls[1], in_=w2lr[1]).then_inc(w2_sems[1], 16)
            sync.dma_start(out=w3s, in_=w3).then_inc(w3_sem, 16)
            sync.dma_start(out=ones_s, in_=one).then_inc(w3_sem, 16)
            dma_x(sync, 1)
            dma_x(sync, 2)
            dma_x(sync, 3)

            out_r = out.rearrange("m (nch ch) -> m nch ch", ch=CH)
            for c in range(XBUF, NCH):
                sync.wait_ge(mm_sem, mmT[("l1", c - XBUF, MPC - 1, MT - 1)])
                dma_x(sync, c)
                cs = c - XBUF
                sync.wait_ge(dve_sem, dveT[("cp", cs)])
                sync.wait_ge(act_sem, actT[("cpa", cs)])
                sync.dma_start(out=out_r[0:1, cs], in_=osb0[:, cs]
                               ).then_inc(d_sem, 16)
                sync.dma_start(out=out_r[1:2, cs], in_=osb1[:, cs]
                               ).then_inc(d_sem, 16)
            for cs in range(NCH - XBUF, NCH):
                sync.wait_ge(dve_sem, dveT[("cp", cs)])
                sync.wait_ge(act_sem, actT[("cpa", cs)])
                sync.dma_start(out=out_r[0:1, cs], in_=osb0[:, cs]
                               ).then_inc(d_sem, 16)
                sync.dma_start(out=out_r[1:2, cs], in_=osb1[:, cs]
                               ).then_inc(d_sem, 16)
            sync.wait_ge(d_sem, 32 * NCH)

        @block.tensor
        def _(tensor: bass.BassEngine):
            for i in range(N_WARM):
                tensor.matmul(psA[:, 0, :], scr[:, :128], scr[:, 128:],
                              start=True, stop=True, skip_group_check=True)

            def dr_group(ps_bank, wh_s, wl_s, hh_ap, hl_ap, msl):
                # 6 DoubleRow matmuls: main x2, x-corr x2, w-corr x2
                planes = []
                for t2 in range(KT2):
                    planes.append((wh_s[:, 2 * t2:2 * t2 + 2, msl],
                                   hh_ap[:, 2 * t2:2 * t2 + 2, :]))
                for t2 in range(KT2):
                    planes.append((wh_s[:, 2 * t2:2 * t2 + 2, msl],
                                   hl_ap[:, 2 * t2:2 * t2 + 2, :]))
                for t2 in range(KT2):
                    planes.append((wl_s[:, 2 * t2:2 * t2 + 2, msl],
                                   hh_ap[:, 2 * t2:2 * t2 + 2, :]))
                n = len(planes)
                for i, (w_ap, m_ap) in enumerate(planes):
                    ins = tensor.matmul(
                        ps_bank, w_ap, m_ap,
                        start=(i == 0), stop=(i == n - 1), perf_mode=DR,
                    )
                return ins

            def l3(cc, m):
                # L3 row for chunk cc -> partition 0 of psB bank m (matmul
                # dst must start at partition 0)
                if m == 0:
                    tensor.wait_ge(pool_sem, poolT[("red0", cc)])
                else:
                    tensor.wait_ge(dve_sem, dveT[("red1", cc)])
                tensor.matmul(
                    psB[0:1, m, :], ones_s, t_r[:, m, cc % 2, :],
                    start=True, stop=True,
                ).then_inc(mm_sem, 1)

            for c in range(NCH):
                tensor.wait_ge(x_sems[c % XBUF], 32 * (c // XBUF + 1))
                for m, mt in ORDER:     # L1
                    if mt == 0:
                        if c == 0:
                            tensor.wait_ge(w1_sems[m], 32)
                        else:
                            # banks 2m/2m+1 drained by c-1's h1h+h1l reads
                            tensor.wait_ge(act_sem,
                                           actT[("r1", c - 1, m, MT - 1)])
                            tensor.wait_ge(dve_sem,
                                           dveT[("h1l", c - 1, m, MT - 1)])
                    if mt >= 2:         # 2-bank rotation WAR
                        tensor.wait_ge(act_sem, actT[("r1", c, m, mt - 2)])
                        tensor.wait_ge(dve_sem, dveT[("h1l", c, m, mt - 2)])
                    msl = slice(mt * 128, (mt + 1) * 128)
                    ins = dr_group(psA[:, 2 * m + mt % 2, :],
                                   w1hs[m], w1ls[m],
                                   xsh[:, c % XBUF], xsl[:, c % XBUF], msl)
                    ins.then_inc(mm_sem, 1)
                for m, mt in ORDER_L2:  # L2
                    if mt == 0:
                        if c == 0:
                            tensor.wait_ge(w2_sems[m], 32)
                        # h1h/h1l of member m complete (also implies the
                        # previous chunk's h2 drains, ACT being in-order)
                        tensor.wait_ge(act_sem, actT[("r1", c, m, MT - 1)])
                        tensor.wait_ge(dve_sem, dveT[("h1l", c, m, MT - 1)])
                        if m == 0 and c >= 3:
                            # bank 0 holds c-3's m0 L3 row until copied out
                            tensor.wait_ge(dve_sem, dveT[("cp", c - 3)])
                    if mt == 1 and m == 0 and c >= 3:
                        # bank 1 holds c-3's m1 L3 row until ACT copies it
                        tensor.wait_ge(act_sem, actT[("cpa", c - 3)])
                    if mt >= 2:
                        tensor.wait_ge(act_sem, actT[("r2", c, m, mt - 2)])
                    msl = slice(mt * 128, (mt + 1) * 128)
                    ins = dr_group(psB[:, 2 * m + mt % 2, :],
                                   w2hs[m], w2ls[m],
                                   h1h[:, m], h1l[:, m], msl)
                    ins.then_inc(mm_sem, 1)
                if c >= 2:
                    # L3 rows of chunk c-2 land in bank 0 once its last L2
                    # writer (m=0, mt=MT-2) has been drained by ACT
                    if c == 2:
                        tensor.wait_ge(w3_sem, 32)
                    tensor.wait_ge(act_sem, actT[("r2", c, 0, MT - 1)])
                    for m in range(MPC):
                        l3(c - 2, m)
            # endgame: ones-matmul L3 for chunk NCH-2, then direct f32r L3
            # for the last chunk straight off h2
            tensor.wait_ge(dve_sem, dveT[("cp", NCH - 3)])
            tensor.wait_ge(act_sem, actT[("cpa", NCH - 3)])
            for m in range(MPC):
                l3(NCH - 2, m)
            for m in range(MPC):
                if m == 0:
                    tensor.wait_ge(act_sem, actT[("r2", NCH - 1, 0, MT - 1)])
                else:
                    tensor.wait_ge(dve_sem, dveT[("h2d", NCH - 1, MT - 1)])
                tensor.wait_ge(dve_sem, dveT[("cp", NCH - 2)])
                tensor.wait_ge(act_sem, actT[("cpa", NCH - 2)])
                for kt in range(KT):
                    ins = tensor.matmul(
                        psB[0:1, m, :],
                        w3s[:, m, kt:kt + 1],
                        h2[:, m, kt, :],
                        start=(kt == 0), stop=(kt == KT - 1),
                    )
                ins.then_inc(mm_sem, 1)

        @block.scalar
        def _(scalar: bass.BassEngine):
            Copy = mybir.ActivationFunctionType.Copy

            def cp_row1(cc):
                # member-1 L3 row (partition 32), in ACT's idle window
                scalar.wait_ge(mm_sem, mmT[("l3", cc, 1)])
                scalar.activation(
                    osb1[0:1, cc, :], psB[0:1, 1, :], Copy,
                ).then_inc(act_sem, 1)

            scalar.wait_ge(b1_sem, 16)
            for c in range(NCH):
                if c >= 3:
                    cp_row1(c - 3)
                for m, mt in ORDER:
                    scalar.wait_ge(mm_sem, mmT[("l1", c, m, mt)])
                    scalar.activation(
                        h1h[:, m, mt, :], psA[:, 2 * m + mt % 2, :], Relu,
                        bias=b1s[:, m, mt:mt + 1],
                    ).then_inc(act_sem, 1)
                if c == 0:
                    scalar.wait_ge(b2_sem, 16)
                for m, mt in ORDER6:
                    scalar.wait_ge(mm_sem, mmT[("l2", c, m, mt)])
                    if c >= 1:
                        # h2[m, mt] still read by Pool muls of chunk c-1
                        key = ("mul0", c - 1, mt) if m == 0 else \
                            ("mul1", c - 1, mt)
                        scalar.wait_ge(pool_sem, poolT[key])
                    scalar.activation(
                        h2[:, m, mt, :], psB[:, 2 * m + mt % 2, :], Relu,
                        bias=b2s[:, m, mt:mt + 1],
                    ).then_inc(act_sem, 1)
            for cc in (NCH - 3, NCH - 2, NCH - 1):
                cp_row1(cc)

        @block.gpsimd
        def _(pool: bass.BassEngine):
            # L3 kt-reductions (SBUF-only; GPSIMD cannot access PSUM):
            # member-0's full chain + member-1's products
            pool.wait_ge(w3_sem, 32)
            for cc in range(NCH - 1):
                pool.wait_ge(act_sem, actT[("r2", cc, 0, 0)])
                pool.tensor_scalar_mul(rA, h2[:, 0, 0, :], w3f[:, 0, 0:1]
                                       ).then_inc(pool_sem, 1)
                pool.wait_ge(act_sem, actT[("r2", cc, 0, 1)])
                pool.tensor_scalar_mul(rB, h2[:, 0, 1, :], w3f[:, 0, 1:2]
                                       ).then_inc(pool_sem, 1)
                pool.tensor_tensor(rE, rA, rB, add_op)
                pool.wait_ge(act_sem, actT[("r2", cc, 0, 2)])
                pool.tensor_scalar_mul(rC, h2[:, 0, 2, :], w3f[:, 0, 2:3]
                                       ).then_inc(pool_sem, 1)
                pool.wait_ge(act_sem, actT[("r2", cc, 0, 3)])
                pool.tensor_scalar_mul(rD, h2[:, 0, 3, :], w3f[:, 0, 3:4]
                                       ).then_inc(pool_sem, 1)
                pool.tensor_tensor(rF, rC, rD, add_op)
                pool.tensor_tensor(t_r[:, 0, cc % 2, :], rE, rF, add_op
                                   ).then_inc(pool_sem, 1)
                if cc >= 1:
                    # sA..sD still read by DVE adds1(cc-1) until sF lands
                    pool.wait_ge(rd2_sem, 2 * cc)
                for kt, buf in enumerate((sA, sB, sC, sD)):
                    if kt < 2:
                        pool.wait_ge(act_sem, actT[("r2", cc, 1, kt)])
                    else:
                        pool.wait_ge(dve_sem, dveT[("h2d", cc, kt)])
                    pool.tensor_scalar_mul(buf, h2[:, 1, kt, :],
                                           w3f[:, 1, kt:kt + 1]
                                           ).then_inc(pool_sem, 1)

        @block.vector
        def _(vector: bass.BassEngine):
            def emit_adds1(cc):
                # member-1 add tree; sA..sD written by Pool. rd2 counts
                # sE2/sF2 (2/group); t_r's completion is its red1 tick
                if cc >= 1:
                    vector.wait_ge(dve_sem, dveT[("red1", cc - 1)])
                vector.wait_ge(pool_sem, poolT[("mul1", cc, 1)])
                vector.tensor_tensor(sE, sA, sB, add_op).then_inc(rd2_sem, 1)
                vector.wait_ge(pool_sem, poolT[("mul1", cc, 3)])
                vector.tensor_tensor(sF, sC, sD, add_op).then_inc(rd2_sem, 1)
                vector.wait_ge(rd2_sem, 2 * cc + 2)
                vector.tensor_tensor(t_r[:, 1, cc % 2, :], sE, sF, add_op
                                     ).then_inc(dve_sem, 1)

            def cp_rows(cc):
                # member-0 L3 row (partition 0); ACT copies member-1's
                vector.wait_ge(mm_sem, mmT[("l3", cc, 0)])
                vector.tensor_copy(
                    osb0[0:1, cc, :], psB[0:1, 0, :],
                ).then_inc(dve_sem, 1)

            vector.memset(zero_s, 0.0)
            for c in range(NCH):
                if c >= 3:
                    cp_rows(c - 3)
                for m, mt in ORDER:
                    vector.wait_ge(act_sem, actT[("r1", c, m, mt)])
                    bank = psA[:, 2 * m + mt % 2, :]
                    if b1_nonzero:
                        vector.tensor_scalar_add(tb, bank,
                                                 b1s[:, m, mt:mt + 1])
                        bank = tb
                    vector.scalar_tensor_tensor(
                        h1l[:, m, mt, :], bank, 0.0, h1h[:, m, mt, :],
                        max_op, sub_op,
                    ).then_inc(dve_sem, 1)
                if c >= 1:
                    emit_adds1(c - 1)
                for kt in (2, 3):
                    # member-1 h2 tail tiles: (psB + b2) max 0, f32r out
                    vector.wait_ge(mm_sem, mmT[("l2", c, 1, kt)])
                    if c >= 1:
                        vector.wait_ge(pool_sem, poolT[("mul1", c - 1, kt)])
                    vector.scalar_tensor_tensor(
                        h2[:, 1, kt, :], psB[:, 2 + kt % 2, :],
                        b2s[:, 1, kt:kt + 1], zero_s, add_op, max_op,
                    ).then_inc(dve_sem, 1)
            for cc in (NCH - 3, NCH - 2, NCH - 1):
                cp_rows(cc)

    return nc


def get_nc():
    key = _CACHE.get("b1_nonzero", False)
    if "nc" not in _CACHE:
        _CACHE["nc"] = _build(key)
    return _CACHE["nc"]


def _split8(a, s):
    """a*s -> (hi fp8, lo fp8) with hi + lo == a*s + O(eps^2)."""
    hi = (a * s).astype(F8NP)
    lo = (a * s - hi.astype(np.float32)).astype(F8NP)
    return hi, lo


def kernel(x, W1, b1, W2, b2, W3, b3):
    from concourse.bass_utils import run_bass_kernel_spmd

    x = np.asarray(x, dtype=np.float32)
    W1 = np.asarray(W1, dtype=np.float32)
    W2 = np.asarray(W2, dtype=np.float32)
    W3 = np.asarray(W3, dtype=np.float32)
    b1 = np.asarray(b1, dtype=np.float32)
    b2 = np.asarray(b2, dtype=np.float32)
    b3 = np.asarray(b3, dtype=np.float32)

    _CACHE["b1_nonzero"] = bool(np.any(b1 != 0))
    nc = get_nc()

    xT = np.ascontiguousarray(x.T)
    xh8, xl8 = _split8(xT, SX)

    def feat_major(v):
        # [MPC, H] -> [128, MPC, H//128]: v[p, m, t] = v_in[m, t*128 + p]
        return np.ascontiguousarray(
            v.reshape(MPC, H // 128, 128).transpose(2, 0, 1))

    in_maps = []
    for cr in range(N_CORES):
        s = slice(MPC * cr, MPC * (cr + 1))
        w1h8, w1l8 = _split8(W1[s], SW)
        w2h8, w2l8 = _split8(W2[s], SW)
        in_maps.append({
            "xh": xh8,
            "xl": xl8,
            "w1h": np.ascontiguousarray(w1h8),
            "w1l": np.ascontiguousarray(w1l8),
            "w2h": np.ascontiguousarray(w2h8),
            "w2l": np.ascontiguousarray(w2l8),
            "w3": feat_major(W3[s, :, 0]) / (SX * SW * SW),
            "b1": feat_major(b1[s]) * (SX * SW),
            "b2": feat_major(b2[s]) * (SX * SW * SW),
            "one": np.ones((128, 1), dtype=np.float32),
        })

    res = run_bass_kernel_spmd(nc, in_maps, list(range(N_CORES)))
    out = np.concatenate([r["out"] for r in res.results], axis=0)  # [E, B]
    out = out + b3.reshape(E, 1)
    return out.reshape(E, B, 1).astype(np.float32)


# revision 3
# speedup vs baseline: 1.2718x; 1.0421x over previous
"""EnsembleFC (E=16 MLPs, 512->512->512->1, relu) on 8 TRN2 NeuronCores.

Expert-parallel: each core owns E/8 = 2 members; x replicated. Activations
stay feature-major (transposed): h^T = relu(W^T @ x^T + b).

L1 and L2 run as fp8(e4m3) DoubleRow matmuls with an error-compensated
3-plane split. Every operand is stored as a high fp8 tensor plus an fp8
residual at the same scale (v*s = vh + vl + O(eps^2)); a logical product
W^T x then needs three fp8 planes -- Wh.xh + Wh.xl + Wl.xh (the Wl.xl term
is eps^2-small and dropped), all sharing product scale sw*sx so they
accumulate into one psum group. DoubleRow packs 2 such 128-deep planes per
instruction at 0.5 cycles/row, so each logical 128x128x512 tile costs 3/4
of an f32r matmul while the measured end-to-end error stays ~5e-3 scaled
(vs 2e-2 budget). x and W splits are quantized on the host; h1's split is
computed on-device: ACT writes h1h = fp8(relu(psum+b1s)), DVE writes
h1l = fp8(max(psum+b1s,0) - h1h) in one scalar_tensor_tensor op.

Scales: x*4, W1*4 -> psum1 = z1*16; h1 stored at scale 16; W2*4 ->
psum2 = z2*64; h2 stored (f32r) at scale 64; w3 host-scaled by /64 so the
L3 reduction lands at true scale. Biases fold in as b1*16 / b2*64 (ACT
bias APs); b3 added on host.

Engine split per chunk of 512 batch columns (PE ~10.7us is the roofline):
  PE:   8 L1 groups (6 DR matmuls each) | 2 L3 ones-matmuls | 8 L2 groups.
        Members' groups interleave in pairs (ORDER) so one member's psum
        drain lands while the other occupies the PE.
  ACT:  h1h = fp8(relu(psA+b1s)); h2 = relu(psB+b2s) in f32r.
  Pool: member-0's whole L3 kt-reduction (products rA..rD via
        tensor_scalar_mul, add tree, t_r[0]) plus member-1's products
        sA..sD. GPSIMD cannot touch PSUM, so it gets all-SBUF work.
  DVE:  h1l residual; member-1's add tree -> t_r[1]; L3 psum->osb copies.
  SP:   weight/x DMAs (64KB slices so chunk-0's x lands in ~4us across
        parallel queues instead of 11us on one), output stores.

The L3 row for chunk c is produced by the PE slot in chunk c+2 (t_r is
double-buffered): the Pool/DVE reduction gets a whole chunk of slack, so
the PE never stalls on it mid-chunk (a 1-chunk pipeline left ~4.8us/chunk
of PE idle waiting for t_r). L3 of the last chunk runs as direct f32r
matmuls on the PE tail. A short burst of dummy matmuls during the DMA
prologue keeps the PE pstate ramp off the critical path (an idle PE
falls back to half clock for 3us, so gaps are doubly expensive).
"""
import numpy as np
import ml_dtypes

F8NP = ml_dtypes.float8_e4m3

E, D, H, B = 16, 512, 512, 8192
N_CORES = 8
MPC = E // N_CORES          # members per core
KT = D // 128               # k-tiles per 512 contraction
KT2 = KT // 2               # DoubleRow k-tile pairs
MT = H // 128               # m-tiles per 512 output dim
CH = 512                    # batch columns per chunk (one psum bank)
NCH = B // CH               # chunks
XBUF = 4                    # x chunk buffering
SX = 4.0                    # x scale
SW = 4.0                    # weight scale (both layers)
N_WARM = 11                 # PE pstate warmup matmuls

_CACHE = {}


def _build(b1_nonzero):
    import concourse.bass as bass
    from concourse import mybir

    f32 = mybir.dt.float32
    f32r = mybir.dt.float32r
    fp8 = mybir.dt.float8e4
    DR = mybir.MatmulPerfMode.DoubleRow
    Relu = mybir.ActivationFunctionType.Relu
    add_op = mybir.AluOpType.add
    sub_op = mybir.AluOpType.subtract
    max_op = mybir.AluOpType.max

    nc = bass.Bass("TRN2", target_bir_lowering=False, debug=False,
                   num_devices=N_CORES)

    xh = nc.dram_tensor("xh", [D, B], fp8, kind="ExternalInput").ap()
    xl = nc.dram_tensor("xl", [D, B], fp8, kind="ExternalInput").ap()
    w1h = nc.dram_tensor("w1h", [MPC, D, H], fp8, kind="ExternalInput").ap()
    w1l = nc.dram_tensor("w1l", [MPC, D, H], fp8, kind="ExternalInput").ap()
    w2h = nc.dram_tensor("w2h", [MPC, H, H], fp8, kind="ExternalInput").ap()
    w2l = nc.dram_tensor("w2l", [MPC, H, H], fp8, kind="ExternalInput").ap()
    # host-side pre-arranged: w3[p, m, kt] = W3/64, b1[p,m,mt]*16, b2*64
    w3 = nc.dram_tensor("w3", [128, MPC, KT], f32r, kind="ExternalInput").ap()
    b1 = nc.dram_tensor("b1", [128, MPC, MT], f32, kind="ExternalInput").ap()
    b2 = nc.dram_tensor("b2", [128, MPC, MT], f32, kind="ExternalInput").ap()
    one = nc.dram_tensor("one", [128, 1], f32r, kind="ExternalInput").ap()
    out = nc.dram_tensor("out", [MPC, B], f32, kind="ExternalOutput").ap()

    w1hs = [nc.alloc_sbuf_tensor(f"w1hs{m}", [128, KT, H], fp8).ap()
            for m in range(MPC)]
    w1ls = [nc.alloc_sbuf_tensor(f"w1ls{m}", [128, KT, H], fp8).ap()
            for m in range(MPC)]
    w2hs = [nc.alloc_sbuf_tensor(f"w2hs{m}", [128, KT, H], fp8).ap()
            for m in range(MPC)]
    w2ls = [nc.alloc_sbuf_tensor(f"w2ls{m}", [128, KT, H], fp8).ap()
            for m in range(MPC)]
    w3s = nc.alloc_sbuf_tensor("w3s", [128, MPC, KT], f32r).ap()
    b1s = nc.alloc_sbuf_tensor("b1s", [128, MPC, MT], f32).ap()
    b2s = nc.alloc_sbuf_tensor("b2s", [128, MPC, MT], f32).ap()
    ones_s = nc.alloc_sbuf_tensor("ones_s", [128, 1], f32r).ap()
    xsh = nc.alloc_sbuf_tensor("xsh", [128, XBUF, KT, CH], fp8).ap()
    xsl = nc.alloc_sbuf_tensor("xsl", [128, XBUF, KT, CH], fp8).ap()
    h1h = nc.alloc_sbuf_tensor("h1h", [128, MPC, KT, CH], fp8).ap()
    h1l = nc.alloc_sbuf_tensor("h1l", [128, MPC, KT, CH], fp8).ap()
    h2 = nc.alloc_sbuf_tensor("h2", [128, MPC, KT, CH], f32r).ap()
    zero_s = nc.alloc_sbuf_tensor("zero_s", [128, CH], f32).ap()
    if b1_nonzero:
        tb = nc.alloc_sbuf_tensor("tb", [128, CH], f32).ap()
    # L3 kt-reduction scratch: member-0 chain entirely on Pool (rA..rF),
    # member-1 products on Pool (sA..sD), add tree on DVE (sE/sF)
    rA = nc.alloc_sbuf_tensor("rA", [128, CH], f32).ap()
    rB = nc.alloc_sbuf_tensor("rB", [128, CH], f32).ap()
    rC = nc.alloc_sbuf_tensor("rC", [128, CH], f32).ap()
    rD = nc.alloc_sbuf_tensor("rD", [128, CH], f32).ap()
    rE = nc.alloc_sbuf_tensor("rE", [128, CH], f32).ap()
    rF = nc.alloc_sbuf_tensor("rF", [128, CH], f32).ap()
    sA = nc.alloc_sbuf_tensor("sA", [128, CH], f32).ap()
    sB = nc.alloc_sbuf_tensor("sB", [128, CH], f32).ap()
    sC = nc.alloc_sbuf_tensor("sC", [128, CH], f32).ap()
    sD = nc.alloc_sbuf_tensor("sD", [128, CH], f32).ap()
    sE = nc.alloc_sbuf_tensor("sE", [128, CH], f32).ap()
    sF = nc.alloc_sbuf_tensor("sF", [128, CH], f32).ap()
    # t_r double-buffered: written while the PE reads the older chunk's
    t_r = nc.alloc_sbuf_tensor("t_r", [128, MPC, 2, CH], f32r).ap()
    # L3 row staging, both members at partition 0 (engine copies cannot
    # shift partitions)
    osb0 = nc.alloc_sbuf_tensor("osb0", [1, NCH, CH], f32).ap()
    osb1 = nc.alloc_sbuf_tensor("osb1", [1, NCH, CH], f32).ap()

    psA = nc.alloc_psum_tensor("psA", [128, 2 * MPC, CH], f32).ap()  # L1
    psB = nc.alloc_psum_tensor("psB", [128, 2 * MPC, CH], f32).ap()  # L2+L3

    # PE warmup scratch (uninitialized on HW -- harmless, psum start=True)
    scr = nc.alloc_sbuf_tensor("scr", [128, 128 + CH], f32r).ap()

    xh_r = xh.rearrange("(kt p) b -> p kt b", p=128)
    xl_r = xl.rearrange("(kt p) b -> p kt b", p=128)
    w3f = w3s.bitcast(f32)

    # --- tick tables (absolute semaphore counts, mirror emission order) ---
    ORDER = [(0, 0), (0, 1), (1, 0), (1, 1), (0, 2), (0, 3), (1, 2), (1, 3)]
    ORDER_L2 = [(0, 0), (0, 1), (1, 0), (1, 1), (0, 2), (0, 3), (1, 2), (1, 3)]

    mmT = {}
    _t = 0
    for c in range(NCH):
        for m, mt in ORDER:
            _t += 1
            mmT[("l1", c, m, mt)] = _t
        for m, mt in ORDER_L2:
            _t += 1
            mmT[("l2", c, m, mt)] = _t
        if c >= 2:
            for m in range(MPC):
                _t += 1
                mmT[("l3", c - 2, m)] = _t
    for m in range(MPC):
        _t += 1
        mmT[("l3", NCH - 2, m)] = _t
    for m in range(MPC):
        _t += 1
        mmT[("l3", NCH - 1, m)] = _t

    # ACT: per chunk: h1h ORDER | h2 for ORDER6 (member-1 kt 2/3 go to DVE
    # so ACT's backlog never straddles the chunk boundary)
    ORDER6 = [om for om in ORDER_L2 if om not in ((1, 2), (1, 3))]
    actT = {}
    _a = 0
    for c in range(NCH):
        if c >= 3:
            _a += 1
            actT[("cpa", c - 3)] = _a
        for m, mt in ORDER:
            _a += 1
            actT[("r1", c, m, mt)] = _a
        for m, mt in ORDER6:
            _a += 1
            actT[("r2", c, m, mt)] = _a
    for cc in (NCH - 3, NCH - 2, NCH - 1):
        _a += 1
        actT[("cpa", cc)] = _a

    # Pool: chunk cc's reduction runs during chunk cc+1:
    # mul0 k0,k1 | rE | mul0 k2,k3 | rF | t_r[0] | mul1 k0..k3
    # (rE/rF don't inc pool_sem -- no external waiters)
    poolT = {}
    _p = 0
    for cc in range(NCH - 1):
        for kt in range(KT):
            _p += 1
            poolT[("mul0", cc, kt)] = _p
        _p += 1
        poolT[("red0", cc)] = _p
        for kt in range(KT):
            _p += 1
            poolT[("mul1", cc, kt)] = _p

    # DVE: per chunk: cp(c-3) (both rows, one strided op) | h1l ORDER |
    # adds1(c-1) -> red1 | h2d(c, kt=2,3) (member-1 tail h2 tiles)
    dveT = {}
    _d = 0
    for c in range(NCH):
        if c >= 3:
            _d += 1
            dveT[("cp", c - 3)] = _d
        for m, mt in ORDER:
            _d += 1
            dveT[("h1l", c, m, mt)] = _d
        if c >= 1:
            _d += 1
            dveT[("red1", c - 1)] = _d
        for kt in (2, 3):
            _d += 1
            dveT[("h2d", c, kt)] = _d
    for cc in (NCH - 3, NCH - 2, NCH - 1):
        _d += 1
        dveT[("cp", cc)] = _d

    with (
        nc.Block() as block,
        nc.semaphore("mm_sem") as mm_sem,
        nc.semaphore("act_sem") as act_sem,
        nc.semaphore("pool_sem") as pool_sem,
        nc.semaphore("b1_sem") as b1_sem,
        nc.semaphore("b2_sem") as b2_sem,
        nc.semaphore("w3_sem") as w3_sem,
        nc.semaphore("d_sem") as d_sem,
    ):
        x_sems = [nc.alloc_semaphore(f"x_sem{s}") for s in range(XBUF)]
        dve_sem = nc.alloc_semaphore("dve_sem")
        rd2_sem = nc.alloc_semaphore("rd2_sem")  # sE/sF/t_r[1] DVE ordering
        w1_sems = [nc.alloc_semaphore(f"w1_sem{m}") for m in range(MPC)]
        w2_sems = [nc.alloc_semaphore(f"w2_sem{m}") for m in range(MPC)]

        def dma_x(sync, c):
            # one dma_start per tensor: issue bandwidth (~625ns serialized
            # descriptor-gen per dma_start) dominates, not transfer time
            sync.dma_start(
                out=xsh[:, c % XBUF], in_=xh_r[:, :, c * CH:(c + 1) * CH],
            ).then_inc(x_sems[c % XBUF], 16)
            sync.dma_start(
                out=xsl[:, c % XBUF], in_=xl_r[:, :, c * CH:(c + 1) * CH],
            ).then_inc(x_sems[c % XBUF], 16)

        @block.sync
        def _(sync: bass.BassEngine):
            w1hr = [w1h[m].rearrange("(kt p) m2 -> p kt m2", p=128)
                    for m in range(MPC)]
            w1lr = [w1l[m].rearrange("(kt p) m2 -> p kt m2", p=128)
                    for m in range(MPC)]
            w2hr = [w2h[m].rearrange("(kt p) m2 -> p kt m2", p=128)
                    for m in range(MPC)]
            w2lr = [w2l[m].rearrange("(kt p) m2 -> p kt m2", p=128)
                    for m in range(MPC)]
            dma_x(sync, 0)
            sync.dma_start(out=b1s, in_=b1).then_inc(b1_sem, 16)
            sync.dma_start(out=b2s, in_=b2).then_inc(b2_sem, 16)
            sync.dma_start(out=w1hs[0], in_=w1hr[0]).then_inc(w1_sems[0], 16)
            sync.dma_start(out=w1ls[0], in_=w1lr[0]).then_inc(w1_sems[0], 16)
            sync.dma_start(out=w1hs[1], in_=w1hr[1]).then_inc(w1_sems[1], 16)
            sync.dma_start(out=w1ls[1], in_=w1lr[1]).then_inc(w1_sems[1], 16)
            sync.dma_start(out=w2hs[0], in_=w2hr[0]).then_inc(w2_sems[0], 16)
            sync.dma_start(out=w2ls[0], in_=w2lr[0]).then_inc(w2_sems[0], 16)
            sync.dma_start(out=w2hs[1], in_=w2hr[1]).then_inc(w2_sems[1], 16)
            sync.dma_start(out=w2ls[1], in_=w2lr[1]).then_inc(w2_sems[1], 16)
            sync.dma_start(out=w3s, in_=w3).then_inc(w3_sem, 16)
            sync.dma_start(out=ones_s, in_=one).then_inc(w3_sem, 16)
            dma_x(sync, 1)
            dma_x(sync, 2)
            dma_x(sync, 3)

            out_r = out.rearrange("m (nch ch) -> m nch ch", ch=CH)
            for c in range(XBUF, NCH):
                sync.wait_ge(mm_sem, mmT[("l1", c - XBUF, MPC - 1, MT - 1)])
                dma_x(sync, c)
                cs = c - XBUF
                sync.wait_ge(dve_sem, dveT[("cp", cs)])
                sync.wait_ge(act_sem, actT[("cpa", cs)])
                sync.dma_start(out=out_r[0:1, cs], in_=osb0[:, cs]
                               ).then_inc(d_sem, 16)
                sync.dma_start(out=out_r[1:2, cs], in_=osb1[:, cs]
                               ).then_inc(d_sem, 16)
            for cs in range(NCH - XBUF, NCH):
                sync.wait_ge(dve_sem, dveT[("cp", cs)])
                sync.wait_ge(act_sem, actT[("cpa", cs)])
                sync.dma_start(out=out_r[0:1, cs], in_=osb0[:, cs]
                               ).then_inc(d_sem, 16)
                sync.dma_start(out=out_r[1:2, cs], in_=osb1[:, cs]
                               ).then_inc(d_sem, 16)
            sync.wait_ge(d_sem, 32 * NCH)

        @block.tensor
        def _(tensor: bass.BassEngine):
            for i in range(N_WARM):
                tensor.matmul(psA[:, 0, :], scr[:, :128], scr[:, 128:],
                              start=True, stop=True, skip_group_check=True)

            def dr_group(ps_bank, wh_s, wl_s, hh_ap, hl_ap, msl,
                         waits=None):
                # 6 DoubleRow matmuls: planes needing only the hi operand
                # first, residual planes last; `waits` maps plane index ->
                # list of (sem, value) gates injected mid-group so the
                # group starts before the full h1h/h1l set exists
                planes = []
                for t2 in range(KT2):
                    planes.append((wh_s[:, 2 * t2:2 * t2 + 2, msl],
                                   hh_ap[:, 2 * t2:2 * t2 + 2, :]))
                    planes.append((wl_s[:, 2 * t2:2 * t2 + 2, msl],
                                   hh_ap[:, 2 * t2:2 * t2 + 2, :]))
                for t2 in range(KT2):
                    planes.append((wh_s[:, 2 * t2:2 * t2 + 2, msl],
                                   hl_ap[:, 2 * t2:2 * t2 + 2, :]))
                n = len(planes)
                for i, (w_ap, m_ap) in enumerate(planes):
                    if waits and i in waits:
                        for sem, val in waits[i]:
                            tensor.wait_ge(sem, val)
                    ins = tensor.matmul(
                        ps_bank, w_ap, m_ap,
                        start=(i == 0), stop=(i == n - 1), perf_mode=DR,
                    )
                return ins

            def l3(cc, m):
                # L3 row for chunk cc -> partition 0 of psB bank m (matmul
                # dst must start at partition 0)
                if m == 0:
                    tensor.wait_ge(pool_sem, poolT[("red0", cc)])
                else:
                    tensor.wait_ge(dve_sem, dveT[("red1", cc)])
                tensor.matmul(
                    psB[0:1, m, :], ones_s, t_r[:, m, cc % 2, :],
                    start=True, stop=True,
                ).then_inc(mm_sem, 1)

            for c in range(NCH):
                tensor.wait_ge(x_sems[c % XBUF], 32 * (c // XBUF + 1))
                for m, mt in ORDER:     # L1
                    if mt == 0:
                        if c == 0:
                            tensor.wait_ge(w1_sems[m], 32)
                        else:
                            # banks 2m/2m+1 drained by c-1's h1h+h1l reads
                            tensor.wait_ge(act_sem,
                                           actT[("r1", c - 1, m, MT - 1)])
                            tensor.wait_ge(dve_sem,
                                           dveT[("h1l", c - 1, m, MT - 1)])
                    if mt >= 2:         # 2-bank rotation WAR
                        tensor.wait_ge(act_sem, actT[("r1", c, m, mt - 2)])
                        tensor.wait_ge(dve_sem, dveT[("h1l", c, m, mt - 2)])
                    msl = slice(mt * 128, (mt + 1) * 128)
                    ins = dr_group(psA[:, 2 * m + mt % 2, :],
                                   w1hs[m], w1ls[m],
                                   xsh[:, c % XBUF], xsl[:, c % XBUF], msl)
                    ins.then_inc(mm_sem, 1)
                for m, mt in ORDER_L2:  # L2
                    if mt == 0:
                        if c == 0:
                            tensor.wait_ge(w2_sems[m], 32)
                        if m == 0 and c >= 3:
                            # bank 0 holds c-3's m0 L3 row until copied out
                            tensor.wait_ge(dve_sem, dveT[("cp", c - 3)])
                    if mt == 1 and m == 0 and c >= 3:
                        # bank 1 holds c-3's m1 L3 row until ACT copies it
                        tensor.wait_ge(act_sem, actT[("cpa", c - 3)])
                    if mt >= 2:
                        tensor.wait_ge(act_sem, actT[("r2", c, m, mt - 2)])
                    msl = slice(mt * 128, (mt + 1) * 128)
                    # note: the mid-group h1h gate also implies the previous
                    # chunk's h2/h2d drains (ACT in-order; DVE h1l likewise)
                    gates = {
                        0: [(act_sem, actT[("r1", c, m, 1)])],
                        2: [(act_sem, actT[("r1", c, m, MT - 1)])],
                        4: [(dve_sem, dveT[("h1l", c, m, 1)])],
                        5: [(dve_sem, dveT[("h1l", c, m, MT - 1)])],
                    }
                    if m == 1 and mt <= 1 and c >= 1:
                        # banks 2/3 still read by DVE's h2d of chunk c-1
                        gates[0].append(
                            (dve_sem, dveT[("h2d", c - 1, mt + 2)]))
                    ins = dr_group(psB[:, 2 * m + mt % 2, :],
                                   w2hs[m], w2ls[m],
                                   h1h[:, m], h1l[:, m], msl, waits=gates)
                    ins.then_inc(mm_sem, 1)
                if c >= 2:
                    # L3 rows of chunk c-2 land in bank 0 once its last L2
                    # writer (m=0, mt=MT-2) has been drained by ACT
                    if c == 2:
                        tensor.wait_ge(w3_sem, 32)
                    tensor.wait_ge(act_sem, actT[("r2", c, 0, MT - 1)])
                    for m in range(MPC):
                        l3(c - 2, m)
            # endgame: ones-matmul L3 for chunk NCH-2, then direct f32r L3
            # for the last chunk straight off h2
            tensor.wait_ge(dve_sem, dveT[("cp", NCH - 3)])
            tensor.wait_ge(act_sem, actT[("cpa", NCH - 3)])
            for m in range(MPC):
                l3(NCH - 2, m)
            for m in range(MPC):
                if m == 0:
                    tensor.wait_ge(act_sem, actT[("r2", NCH - 1, 0, MT - 1)])
                else:
                    tensor.wait_ge(dve_sem, dveT[("h2d", NCH - 1, MT - 1)])
                tensor.wait_ge(dve_sem, dveT[("cp", NCH - 2)])
                tensor.wait_ge(act_sem, actT[("cpa", NCH - 2)])
                for kt in range(KT):
                    ins = tensor.matmul(
                        psB[0:1, m, :],
                        w3s[:, m, kt:kt + 1],
                        h2[:, m, kt, :],
                        start=(kt == 0), stop=(kt == KT - 1),
                    )
                ins.then_inc(mm_sem, 1)

        @block.scalar
        def _(scalar: bass.BassEngine):
            Copy = mybir.ActivationFunctionType.Copy

            def cp_row1(cc):
                # member-1 L3 row (partition 32), in ACT's idle window
                scalar.wait_ge(mm_sem, mmT[("l3", cc, 1)])
                scalar.activation(
                    osb1[0:1, cc, :], psB[0:1, 1, :], Copy,
                ).then_inc(act_sem, 1)

            scalar.wait_ge(b1_sem, 16)
            for c in range(NCH):
                if c >= 3:
                    cp_row1(c - 3)
                for m, mt in ORDER:
                    scalar.wait_ge(mm_sem, mmT[("l1", c, m, mt)])
                    scalar.activation(
                        h1h[:, m, mt, :], psA[:, 2 * m + mt % 2, :], Relu,
                        bias=b1s[:, m, mt:mt + 1],
                    ).then_inc(act_sem, 1)
                if c == 0:
                    scalar.wait_ge(b2_sem, 16)
                for m, mt in ORDER6:
                    scalar.wait_ge(mm_sem, mmT[("l2", c, m, mt)])
                    if c >= 1:
                        # h2[m, mt] still read by Pool muls of chunk c-1
                        key = ("mul0", c - 1, mt) if m == 0 else \
                            ("mul1", c - 1, mt)
                        scalar.wait_ge(pool_sem, poolT[key])
                    scalar.activation(
                        h2[:, m, mt, :], psB[:, 2 * m + mt % 2, :], Relu,
                        bias=b2s[:, m, mt:mt + 1],
                    ).then_inc(act_sem, 1)
            for cc in (NCH - 3, NCH - 2, NCH - 1):
                cp_row1(cc)

        @block.gpsimd
        def _(pool: bass.BassEngine):
            # L3 kt-reductions (SBUF-only; GPSIMD cannot access PSUM):
            # member-0's full chain + member-1's products
            pool.wait_ge(w3_sem, 32)
            for cc in range(NCH - 1):
                pool.wait_ge(act_sem, actT[("r2", cc, 0, 0)])
                pool.tensor_scalar_mul(rA, h2[:, 0, 0, :], w3f[:, 0, 0:1]
                                       ).then_inc(pool_sem, 1)
                pool.wait_ge(act_sem, actT[("r2", cc, 0, 1)])
                pool.tensor_scalar_mul(rB, h2[:, 0, 1, :], w3f[:, 0, 1:2]
                                       ).then_inc(pool_sem, 1)
                pool.tensor_tensor(rE, rA, rB, add_op)
                pool.wait_ge(act_sem, actT[("r2", cc, 0, 2)])
                pool.tensor_scalar_mul(rC, h2[:, 0, 2, :], w3f[:, 0, 2:3]
                                       ).then_inc(pool_sem, 1)
                pool.wait_ge(act_sem, actT[("r2", cc, 0, 3)])
                pool.tensor_scalar_mul(rD, h2[:, 0, 3, :], w3f[:, 0, 3:4]
                                       ).then_inc(pool_sem, 1)
                pool.tensor_tensor(rF, rC, rD, add_op)
                pool.tensor_tensor(t_r[:, 0, cc % 2, :], rE, rF, add_op
                                   ).then_inc(pool_sem, 1)
                if cc >= 1:
                    # sA..sD still read by DVE adds1(cc-1) until sF lands
                    pool.wait_ge(rd2_sem, 2 * cc)
                for kt, buf in enumerate((sA, sB, sC, sD)):
                    if kt < 2:
                        pool.wait_ge(act_sem, actT[("r2", cc, 1, kt)])
                    else:
                        pool.wait_ge(dve_sem, dveT[("h2d", cc, kt)])
                    pool.tensor_scalar_mul(buf, h2[:, 1, kt, :],
                                           w3f[:, 1, kt:kt + 1]
                                           ).then_inc(pool_sem, 1)

        @block.vector
        def _(vector: bass.BassEngine):
            def emit_adds1(cc):
                # member-1 add tree; sA..sD written by Pool. rd2 counts
                # sE2/sF2 (2/group); t_r's completion is its red1 tick
                if cc >= 1:
                    vector.wait_ge(dve_sem, dveT[("red1", cc - 1)])
                vector.wait_ge(pool_sem, poolT[("mul1", cc, 1)])
                vector.tensor_tensor(sE, sA, sB, add_op).then_inc(rd2_sem, 1)
                vector.wait_ge(pool_sem, poolT[("mul1", cc, 3)])
                vector.tensor_tensor(sF, sC, sD, add_op).then_inc(rd2_sem, 1)
                vector.wait_ge(rd2_sem, 2 * cc + 2)
                vector.tensor_tensor(t_r[:, 1, cc % 2, :], sE, sF, add_op
                                     ).then_inc(dve_sem, 1)

            def cp_rows(cc):
                # member-0 L3 row (partition 0); ACT copies member-1's
                vector.wait_ge(mm_sem, mmT[("l3", cc, 0)])
                vector.tensor_copy(
                    osb0[0:1, cc, :], psB[0:1, 0, :],
                ).then_inc(dve_sem, 1)

            vector.memset(zero_s, 0.0)
            for c in range(NCH):
                if c >= 3:
                    cp_rows(c - 3)
                for m, mt in ORDER:
                    vector.wait_ge(act_sem, actT[("r1", c, m, mt)])
                    bank = psA[:, 2 * m + mt % 2, :]
                    if b1_nonzero:
                        vector.tensor_scalar_add(tb, bank,
                                                 b1s[:, m, mt:mt + 1])
                        bank = tb
                    vector.scalar_tensor_tensor(
                        h1l[:, m, mt, :], bank, 0.0, h1h[:, m, mt, :],
                        max_op, sub_op,
                    ).then_inc(dve_sem, 1)
                if c >= 1:
                    emit_adds1(c - 1)
                for kt in (2, 3):
                    # member-1 h2 tail tiles: (psB + b2) max 0, f32r out
                    vector.wait_ge(mm_sem, mmT[("l2", c, 1, kt)])
                    if c >= 1:
                        vector.wait_ge(pool_sem, poolT[("mul1", c - 1, kt)])
                    vector.scalar_tensor_tensor(
                        h2[:, 1, kt, :], psB[:, 2 + kt % 2, :],
                        b2s[:, 1, kt:kt + 1], zero_s, add_op, max_op,
                    ).then_inc(dve_sem, 1)
            for cc in (NCH - 3, NCH - 2, NCH - 1):
                cp_rows(cc)

    return nc


def get_nc():
    key = _CACHE.get("b1_nonzero", False)
    if "nc" not in _CACHE:
        _CACHE["nc"] = _build(key)
    return _CACHE["nc"]


def _split8(a, s):
    """a*s -> (hi fp8, lo fp8) with hi + lo == a*s + O(eps^2)."""
    hi = (a * s).astype(F8NP)
    lo = (a * s - hi.astype(np.float32)).astype(F8NP)
    return hi, lo


def kernel(x, W1, b1, W2, b2, W3, b3):
    from concourse.bass_utils import run_bass_kernel_spmd

    x = np.asarray(x, dtype=np.float32)
    W1 = np.asarray(W1, dtype=np.float32)
    W2 = np.asarray(W2, dtype=np.float32)
    W3 = np.asarray(W3, dtype=np.float32)
    b1 = np.asarray(b1, dtype=np.float32)
    b2 = np.asarray(b2, dtype=np.float32)
    b3 = np.asarray(b3, dtype=np.float32)

    _CACHE["b1_nonzero"] = bool(np.any(b1 != 0))
    nc = get_nc()

    xT = np.ascontiguousarray(x.T)
    xh8, xl8 = _split8(xT, SX)

    def feat_major(v):
        # [MPC, H] -> [128, MPC, H//128]: v[p, m, t] = v_in[m, t*128 + p]
        return np.ascontiguousarray(
            v.reshape(MPC, H // 128, 128).transpose(2, 0, 1))

    in_maps = []
    for cr in range(N_CORES):
        s = slice(MPC * cr, MPC * (cr + 1))
        w1h8, w1l8 = _split8(W1[s], SW)
        w2h8, w2l8 = _split8(W2[s], SW)
        in_maps.append({
            "xh": xh8,
            "xl": xl8,
            "w1h": np.ascontiguousarray(w1h8),
            "w1l": np.ascontiguousarray(w1l8),
            "w2h": np.ascontiguousarray(w2h8),
            "w2l": np.ascontiguousarray(w2l8),
            "w3": feat_major(W3[s, :, 0]) / (SX * SW * SW),
            "b1": feat_major(b1[s]) * (SX * SW),
            "b2": feat_major(b2[s]) * (SX * SW * SW),
            "one": np.ones((128, 1), dtype=np.float32),
        })

    res = run_bass_kernel_spmd(nc, in_maps, list(range(N_CORES)))
    out = np.concatenate([r["out"] for r in res.results], axis=0)  # [E, B]
    out = out + b3.reshape(E, 1)
    return out.reshape(E, B, 1).astype(np.float32)
